# revision 5
# baseline (speedup 1.0000x reference)
"""Causal multi-head attention (B=2, T=4096, D=1024, H=16, HD=64) on 8 trn2
NeuronCores.

Sharding: core c handles batch b = c//4 and head group g = c%4 (heads
4g..4g+3).  Each core computes qkv projection for its 4 heads, causal
flash-attention in transposed (S^T) layout, and a partial out-projection
(its 256 columns of the hidden dim).  Host sums the 4 partial outputs per
batch and adds the bias terms.

v2: fully pipelined per-512-query-tile structure.  For each tile j:
qk/v projection -> attention i-loop (QK matmul, exp on ACT, PV matmul
with a ones-column for softmax denominators) -> inline normalization
(reciprocal on PSUM partition 64 + matmul broadcast, fused into the
PSUM-evacuation multiply) -> head-pair restack (odd head of each pair
moved to SBUF partitions 64-127 by DMA) -> out-projection with K=128
-> y DMA.  Everything after the i-loop for tile j overlaps the i-loop
of tile j+1; no DRAM round-trip for the softmax denominators.

Math notes:
  - k-bias kept (cheap), v-bias folded into the host epilogue: softmax
    rows sum to 1, so out += b_v exactly, hence y += b_v @ w_out (+ b_out).
  - softmax computed without max subtraction (scores are O(10) for this
    problem scale; exp stays in fp32 range).
  - softmax denominators come for free as a 65th ones-column in v.
dtypes: q/k path float32r (TF32-like, ~1e-4), P and v bf16, accum fp32.
"""

import numpy as np

import concourse.bass as bass
import concourse.mybir as mybir
import concourse.tile as tile
from concourse import bacc
from concourse.bass_utils import run_bass_kernel_spmd
from concourse.masks import make_upper_triangular

F32 = mybir.dt.float32
F32R = mybir.dt.float32r
BF16 = mybir.dt.bfloat16
AF = mybir.ActivationFunctionType

B, D, H, HD = 2, 1024, 16, 64
NHEADS = 4          # heads per core
SCALE = 1.0 / np.sqrt(HD)


def build(T=4096, reps=1):
    """Build the per-core Bass module. reps>1 wraps the compute in an
    on-device For_i loop (for wall-clock-difference timing)."""
    NJ = T // 512       # tq tiles of 512
    NT = T // 128       # t chunks of 128
    DC = D // 128       # d chunks of 128

    nc = bacc.Bacc("TRN2", target_bir_lowering=False, debug=False, num_devices=8)

    xt_d = nc.dram_tensor("xt", [D, T], F32R, kind="ExternalInput")
    wqk_d = nc.dram_tensor("wqk", [D, 512], F32R, kind="ExternalInput")
    wv_d = nc.dram_tensor("wv", [D, 256], F32R, kind="ExternalInput")
    bqk_d = nc.dram_tensor("bqk", [128, 4], F32, kind="ExternalInput")
    wout_d = nc.dram_tensor("wout", [128, 2, D], F32R, kind="ExternalInput")
    y_d = nc.dram_tensor("y", [T, D], F32, kind="ExternalOutput")

    with tile.TileContext(nc) as tc:
        with (
            tc.tile_pool(name="const", bufs=1) as cp,
            tc.tile_pool(name="persist", bufs=1) as pp,
        ):
            # ---------- constants / weights (outside the timing loop)
            tri32 = cp.tile([128, 128], F32, tag="tri32")
            make_upper_triangular(nc, tri32[:], val=1.0, diag=True)
            tri = cp.tile([128, 128], BF16, tag="tri")
            nc.vector.tensor_copy(tri[:], tri32[:])

            # ones row on partition 64 (for the denominator broadcast matmul)
            ones65f = cp.tile([65, 64], F32, tag="ones65f")
            nc.vector.memset(ones65f[64:65, :], 1.0)
            ones65 = cp.tile([65, 64], F32R, tag="ones65")
            nc.vector.tensor_copy(ones65[64:65, :], ones65f[64:65, :])

            bqk_sb = cp.tile([128, 4], F32, tag="bqk")
            nc.sync.dma_start(bqk_sb[:], bqk_d[:])
            wqk_sb = cp.tile([128, DC, 512], F32R, tag="wqk")
            nc.sync.dma_start(wqk_sb[:], wqk_d.rearrange("(dc p) c -> p dc c", p=128))
            wv_sb = cp.tile([128, DC, 256], F32R, tag="wv")
            nc.sync.dma_start(wv_sb[:], wv_d.rearrange("(dc p) c -> p dc c", p=128))
            wout_sb = cp.tile([128, 2, D], F32R, tag="wout")
            nc.sync.dma_start(wout_sb[:], wout_d[:])

            # ---------- persistent state
            kT = pp.tile([128, 2, T], F32R, tag="kT")          # [qk-col, pair, t]
            v_sb = pp.tile([128, NT, 4, 65], BF16, tag="v")    # [t%128, tchunk, head, hd+one]
            nc.vector.memset(v_sb[:, :, :, 64:65], 1.0)

            def body():
                with (
                    tc.tile_pool(name="work2", bufs=2) as wp2,
                    tc.tile_pool(name="work3", bufs=3) as wp3,
                    tc.tile_pool(name="norm", bufs=2) as npool,
                    tc.tile_pool(name="yout", bufs=3) as ypool,
                    tc.tile_pool(name="pmisc", bufs=2, space="PSUM") as ps_m,
                    tc.tile_pool(name="pscore", bufs=2, space="PSUM") as ps_s,
                    tc.tile_pool(name="pout", bufs=1, space="PSUM") as ps_o,
                ):
                    for j in range(NJ):
                        t0 = 512 * j
                        # ---- load xT columns for rows [t0, t0+512)
                        #      (x is pre-transposed on the host)
                        xT = wp2.tile([128, DC, 512], F32R, tag="xT")
                        nc.sync.dma_start(
                            xT[:],
                            xt_d[:, t0 : t0 + 512].rearrange(
                                "(dc p) t -> p dc t", p=128
                            ),
                        )

                        # ---- project q,k for this tq tile (4 col-chunks)
                        qTj = wp2.tile([128, 2, 512], F32R, tag="qTj")
                        for cc in range(4):
                            pqk = ps_m.tile([128, 512], F32, tag="m")
                            for dc in range(DC):
                                nc.tensor.matmul(
                                    pqk[:],
                                    wqk_sb[:, dc, 128 * cc : 128 * (cc + 1)],
                                    xT[:, dc],
                                    start=(dc == 0),
                                    stop=(dc == DC - 1),
                                )
                            dst = qTj[:, cc] if cc < 2 else kT[:, cc - 2, t0 : t0 + 512]
                            nc.vector.tensor_scalar_add(dst, pqk[:], bqk_sb[:, cc : cc + 1])

                        # ---- project v for this tq tile
                        for ts in range(4):
                            pv = ps_m.tile([128, 256], F32, tag="m")
                            for dc in range(DC):
                                nc.tensor.matmul(
                                    pv[:],
                                    xT[:, dc, 128 * ts : 128 * (ts + 1)],
                                    wv_sb[:, dc],
                                    start=(dc == 0),
                                    stop=(dc == DC - 1),
                                )
                            nc.vector.tensor_copy(
                                v_sb[:, 4 * j + ts, :, 0:64],
                                pv[:].rearrange("p (h c) -> p h c", h=4),
                            )

                        # ---- causal attention for tq tile j, both head pairs
                        #      outN: [128, pair, 512] normalized attn output;
                        #      partitions 0-63 = head 2p, 64-127 = head 2p+1.
                        outN = wp2.tile([128, 2, 512], F32R, tag="outN")
                        nchunk = 4 * (j + 1)
                        for hp in range(2):
                            psO = ps_o.tile([65, 1024], F32, tag="po")
                            for i in range(nchunk):
                                dlt = 128 * i - 512 * j
                                dlt = dlt if dlt > 0 else 0
                                pS = ps_s.tile([128, 1024], F32, tag="ps")
                                for hh in range(2):
                                    nc.tensor.matmul(
                                        pS[:, 512 * hh + dlt : 512 * (hh + 1)],
                                        kT[64 * hh : 64 * (hh + 1), hp, 128 * i : 128 * (i + 1)],
                                        qTj[64 * hh : 64 * (hh + 1), hp, dlt:512],
                                        start=True,
                                        stop=True,
                                    )
                                pT = wp3.tile([128, 2, 512], BF16, tag="pT", bufs=3)
                                pSv = pS[:].rearrange("p (h w) -> p h w", h=2)
                                nc.scalar.activation(
                                    pT[:, :, dlt:512], pSv[:, :, dlt:512], AF.Exp, scale=SCALE
                                )
                                if i >= 4 * j:  # diagonal block: causal 0/1 mask
                                    for hh in range(2):
                                        nc.vector.tensor_tensor(
                                            pT[:, hh, dlt : dlt + 128],
                                            pT[:, hh, dlt : dlt + 128],
                                            tri[:],
                                            mybir.AluOpType.mult,
                                        )
                                for hh in range(2):
                                    nc.tensor.matmul(
                                        psO[0:65, 512 * hh + dlt : 512 * (hh + 1)],
                                        v_sb[:, i, 2 * hp + hh, :],
                                        pT[:, hh, dlt:512],
                                        start=(i == 0),
                                        stop=(i == nchunk - 1),
                                        skip_group_check=True,
                                    )

                            # ---- inline normalization: recip of the ones-row
                            #      sums (partition 64), broadcast down 64
                            #      partitions via matmul, multiply during the
                            #      PSUM evacuation.
                            rr = npool.tile([65, 1024], F32R, tag="rr")
                            with nc.allow_low_precision(
                                reason="tf32 reciprocal of softmax denom (~1e-3)"
                            ):
                                nc.vector.reciprocal(rr[64:65, :], psO[64:65, :])
                            for hh in range(2):
                                pB = ps_m.tile([64, 512], F32, tag="m")
                                nc.tensor.matmul(
                                    pB[:],
                                    ones65[64:65, :],
                                    rr[64:65, 512 * hh : 512 * (hh + 1)],
                                    start=True,
                                    stop=True,
                                )
                                pBs = npool.tile([64, 512], F32, tag="pbs")
                                nc.vector.tensor_copy(pBs[:], pB[:])
                                if hh == 0:
                                    nc.vector.tensor_tensor(
                                        outN[0:64, hp, :],
                                        psO[0:64, 0:512],
                                        pBs[:],
                                        mybir.AluOpType.mult,
                                    )
                                else:
                                    stage = npool.tile([64, 512], F32R, tag="stage")
                                    nc.vector.tensor_tensor(
                                        stage[:],
                                        psO[0:64, 512:1024],
                                        pBs[:],
                                        mybir.AluOpType.mult,
                                    )
                                    # odd head of the pair -> partitions 64-127
                                    nc.sync.dma_start(outN[64:128, hp, :], stage[:])

                        # ---- out-projection for this tile (K=128: head pair
                        #      stacked on partitions) + y writeback
                        for nh in range(2):
                            for tt in range(4):
                                pY = ps_m.tile([128, 512], F32, tag="m")
                                for p in range(2):
                                    nc.tensor.matmul(
                                        pY[:],
                                        outN[:, p, 128 * tt : 128 * (tt + 1)],
                                        wout_sb[:, p, 512 * nh : 512 * (nh + 1)],
                                        start=(p == 0),
                                        stop=(p == 1),
                                    )
                                ysb = ypool.tile([128, 512], F32, tag="y")
                                nc.vector.tensor_copy(ysb[:], pY[:])
                                nc.sync.dma_start(
                                    y_d[
                                        t0 + 128 * tt : t0 + 128 * (tt + 1),
                                        512 * nh : 512 * (nh + 1),
                                    ],
                                    ysb[:],
                                )

            if reps == 1:
                body()
            else:
                with tc.For_i(0, reps, 1):
                    body()

    nc.compile()
    return nc


def shard_inputs(x, w_qkv, b_qkv, w_out, T):
    """Build the 8 per-core input maps (core c: batch c//4, head group c%4)."""
    x = np.asarray(x, dtype=np.float32)
    w_qkv = np.asarray(w_qkv, dtype=np.float32)
    b_qkv = np.asarray(b_qkv, dtype=np.float32)
    w_out = np.asarray(w_out, dtype=np.float32)
    in_maps = []
    for c in range(8):
        b, g = c // 4, c % 4
        qcols = slice(4 * g * 64, (4 * g + 4) * 64)
        kcols = slice(D + 4 * g * 64, D + (4 * g + 4) * 64)
        vcols = slice(2 * D + 4 * g * 64, 2 * D + (4 * g + 4) * 64)
        wqk = np.concatenate([w_qkv[:, qcols], w_qkv[:, kcols]], axis=1)  # [D, 512]
        wv = np.ascontiguousarray(w_qkv[:, vcols])  # [D, 256]
        bqk = np.concatenate([b_qkv[qcols], b_qkv[kcols]]).reshape(4, 128).T  # [128,4]
        # [hidden 256] -> [pair p, in-pair ip, hd], stacked so partitions
        # 0-63 = head 2p, 64-127 = head 2p+1
        w4 = w_out[256 * g : 256 * (g + 1), :].reshape(2, 2, 64, D)
        wout = np.ascontiguousarray(w4.transpose(1, 2, 0, 3).reshape(128, 2, D))
        in_maps.append(
            {
                "xt": np.ascontiguousarray(x[b, :T].T),
                "wqk": np.ascontiguousarray(wqk),
                "wv": wv,
                "bqk": np.ascontiguousarray(bqk),
                "wout": wout,
            }
        )
    return in_maps


def assemble_output(results, b_qkv, b_out, w_out, T):
    b_qkv = np.asarray(b_qkv, dtype=np.float32)
    b_out = np.asarray(b_out, dtype=np.float32)
    w_out = np.asarray(w_out, dtype=np.float32)
    extra = b_out + b_qkv[2 * D :] @ w_out  # v-bias folds through softmax
    y = np.zeros((B, T, D), dtype=np.float32)
    for c in range(8):
        y[c // 4] += results[c]["y"]
    y += extra[None, None, :]
    return y


_cache = {}


def kernel(x, w_qkv, b_qkv, w_out, b_out):
    x = np.asarray(x, dtype=np.float32)
    T = x.shape[1]
    if T not in _cache:
        _cache[T] = build(T=T, reps=1)
    nc = _cache[T]
    in_maps = shard_inputs(x, w_qkv, b_qkv, w_out, T)
    for _attempt in range(3):
        res = run_bass_kernel_spmd(nc, in_maps, core_ids=list(range(8)), trace=False)
        y = assemble_output(res.results, b_qkv, b_out, w_out, T)
        if np.isfinite(y).all():  # guard against transient device flakes
            return y
    return y


# revision 8
# speedup vs baseline: 1.1764x; 1.1764x over previous
"""Causal multi-head attention (B=2, T=4096, D=1024, H=16, HD=64) on 8 trn2
NeuronCores.

Sharding: core c handles batch b = c//4 and head group g = c%4 (heads
4g..4g+3).  Each core computes qkv projection for its 4 heads, causal
flash-attention in transposed (S^T) layout, and a partial out-projection
(its 256 columns of the hidden dim).  Host sums the 4 partial outputs per
batch and adds the bias terms.

v2: fully pipelined per-512-query-tile structure.  For each tile j:
qk/v projection -> attention i-loop (QK matmul, exp on ACT, PV matmul
with a ones-column for softmax denominators) -> inline normalization
(reciprocal on PSUM partition 64 + matmul broadcast, fused into the
PSUM-evacuation multiply) -> head-pair restack (odd head of each pair
moved to SBUF partitions 64-127 by DMA) -> out-projection with K=128
-> y DMA.  Everything after the i-loop for tile j overlaps the i-loop
of tile j+1; no DRAM round-trip for the softmax denominators.

Math notes:
  - k-bias kept (cheap), v-bias folded into the host epilogue: softmax
    rows sum to 1, so out += b_v exactly, hence y += b_v @ w_out (+ b_out).
  - softmax computed without max subtraction (scores are O(10) for this
    problem scale; exp stays in fp32 range).
  - softmax denominators come for free as a 65th ones-column in v.
dtypes: q/k path float32r (TF32-like, ~1e-4), P and v bf16, accum fp32.
"""

import numpy as np

import concourse.bass as bass
import concourse.mybir as mybir
import concourse.tile as tile
from concourse import bacc
from concourse.bass_utils import run_bass_kernel_spmd
from concourse.masks import make_upper_triangular

F32 = mybir.dt.float32
F32R = mybir.dt.float32r
BF16 = mybir.dt.bfloat16
AF = mybir.ActivationFunctionType

B, D, H, HD = 2, 1024, 16, 64
NHEADS = 4          # heads per core
SCALE = 1.0 / np.sqrt(HD)


def build(T=4096, reps=1):
    """Build the per-core Bass module. reps>1 wraps the compute in an
    on-device For_i loop (for wall-clock-difference timing)."""
    NJ = T // 512       # tq tiles of 512
    NT = T // 128       # t chunks of 128
    DC = D // 128       # d chunks of 128

    nc = bacc.Bacc("TRN2", target_bir_lowering=False, debug=False, num_devices=8)

    xt_d = nc.dram_tensor("xt", [D, T], F32R, kind="ExternalInput")
    wqk_d = nc.dram_tensor("wqk", [D, 512], F32R, kind="ExternalInput")
    wv_d = nc.dram_tensor("wv", [D, 256], F32R, kind="ExternalInput")
    bqk_d = nc.dram_tensor("bqk", [128, 4], F32, kind="ExternalInput")
    wout_d = nc.dram_tensor("wout", [128, 2, D], F32R, kind="ExternalInput")
    y_d = nc.dram_tensor("y", [T, D], F32, kind="ExternalOutput")

    with tile.TileContext(nc) as tc:
        with (
            tc.tile_pool(name="const", bufs=1) as cp,
            tc.tile_pool(name="persist", bufs=1) as pp,
        ):
            # ---------- constants / weights (outside the timing loop)
            tri32 = cp.tile([128, 128], F32, tag="tri32")
            make_upper_triangular(nc, tri32[:], val=1.0, diag=True)
            tri = cp.tile([128, 128], BF16, tag="tri")
            nc.vector.tensor_copy(tri[:], tri32[:])

            # ones row on partition 64 (for the denominator broadcast matmul)
            ones65f = cp.tile([65, 64], F32, tag="ones65f")
            nc.vector.memset(ones65f[64:65, :], 1.0)
            ones65 = cp.tile([65, 64], F32R, tag="ones65")
            nc.vector.tensor_copy(ones65[64:65, :], ones65f[64:65, :])

            bqk_sb = cp.tile([128, 4], F32, tag="bqk")
            nc.sync.dma_start(bqk_sb[:], bqk_d[:])
            wqk_sb = cp.tile([128, DC, 512], F32R, tag="wqk")
            nc.sync.dma_start(wqk_sb[:], wqk_d.rearrange("(dc p) c -> p dc c", p=128))
            wv_sb = cp.tile([128, DC, 256], F32R, tag="wv")
            nc.sync.dma_start(wv_sb[:], wv_d.rearrange("(dc p) c -> p dc c", p=128))
            wout_sb = cp.tile([128, 2, D], F32R, tag="wout")
            nc.sync.dma_start(wout_sb[:], wout_d[:])

            # ---------- persistent state
            kT = pp.tile([128, 2, T], F32R, tag="kT")          # [qk-col, pair, t]
            v_sb = pp.tile([128, NT, 4, 65], BF16, tag="v")    # [t%128, tchunk, head, hd+one]
            nc.vector.memset(v_sb[:, :, :, 64:65], 1.0)

            def body():
                with (
                    tc.tile_pool(name="work2", bufs=2) as wp2,
                    tc.tile_pool(name="work3", bufs=3) as wp3,
                    tc.tile_pool(name="norm", bufs=2) as npool,
                    tc.tile_pool(name="yout", bufs=3) as ypool,
                    tc.tile_pool(name="pmisc", bufs=2, space="PSUM") as ps_m,
                    tc.tile_pool(name="pscore", bufs=2, space="PSUM") as ps_s,
                    tc.tile_pool(name="pout", bufs=1, space="PSUM") as ps_o,
                ):
                    # ---- filler units: projection of tile jn (emitted
                    #      interleaved into the previous tile's attention
                    #      i-loop so PE fills its ACT-bound slack and the
                    #      next tile's QK is ready immediately)
                    def load_xT(jn):
                        xT = wp2.tile([128, DC, 512], F32R, tag="xT")
                        nc.sync.dma_start(
                            xT[:],
                            xt_d[:, 512 * jn : 512 * jn + 512].rearrange(
                                "(dc p) t -> p dc t", p=128
                            ),
                        )
                        return xT

                    def proj_qk_unit(jn, cc, xT, qT):
                        pqk = ps_m.tile([128, 512], F32, tag="m")
                        for dc in range(DC):
                            nc.tensor.matmul(
                                pqk[:],
                                wqk_sb[:, dc, 128 * cc : 128 * (cc + 1)],
                                xT[:, dc],
                                start=(dc == 0),
                                stop=(dc == DC - 1),
                            )
                        dst = (
                            qT[:, cc]
                            if cc < 2
                            else kT[:, cc - 2, 512 * jn : 512 * jn + 512]
                        )
                        nc.vector.tensor_scalar_add(dst, pqk[:], bqk_sb[:, cc : cc + 1])

                    def proj_v_unit(jn, ts, xT):
                        pv = ps_m.tile([128, 256], F32, tag="m")
                        for dc in range(DC):
                            nc.tensor.matmul(
                                pv[:],
                                xT[:, dc, 128 * ts : 128 * (ts + 1)],
                                wv_sb[:, dc],
                                start=(dc == 0),
                                stop=(dc == DC - 1),
                            )
                        nc.vector.tensor_copy(
                            v_sb[:, 4 * jn + ts, :, 0:64],
                            pv[:].rearrange("p (h c) -> p h c", h=4),
                        )

                    def outproj_unit(jp, tt, outN_p):
                        ysb = ypool.tile([128, 2, 512], F32, tag="y")
                        for nh in range(2):
                            pY = ps_m.tile([128, 512], F32, tag="m")
                            for p in range(2):
                                nc.tensor.matmul(
                                    pY[:],
                                    outN_p[:, p, 128 * tt : 128 * (tt + 1)],
                                    wout_sb[:, p, 512 * nh : 512 * (nh + 1)],
                                    start=(p == 0),
                                    stop=(p == 1),
                                )
                            nc.vector.tensor_copy(ysb[:, nh, :], pY[:])
                        nc.sync.dma_start(
                            y_d[512 * jp + 128 * tt : 512 * jp + 128 * (tt + 1), :],
                            ysb[:].rearrange("p a b -> p (a b)"),
                        )

                    # ---- prologue: projection for tile 0
                    xT0 = load_xT(0)
                    qT_cur = wp2.tile([128, 2, 512], F32R, tag="qTj")
                    for cc in range(4):
                        proj_qk_unit(0, cc, xT0, qT_cur)
                    for ts in range(4):
                        proj_v_unit(0, ts, xT0)

                    outN_prev = None
                    for j in range(NJ):
                        qTj = qT_cur
                        # build the filler list for this tile's attention:
                        # projection of tile j+1, then out-projection of j-1
                        fillers = []
                        if j + 1 < NJ:
                            xTn = load_xT(j + 1)
                            qT_nxt = wp2.tile([128, 2, 512], F32R, tag="qTj")
                            for cc in range(4):
                                fillers.append(
                                    lambda cc=cc, x=xTn, q=qT_nxt: proj_qk_unit(
                                        j + 1, cc, x, q
                                    )
                                )
                            for ts in range(4):
                                fillers.append(
                                    lambda ts=ts, x=xTn: proj_v_unit(j + 1, ts, x)
                                )
                        else:
                            qT_nxt = None
                        if j >= 1:
                            for tt in range(4):
                                fillers.append(
                                    lambda tt=tt, o=outN_prev, jp=j - 1: outproj_unit(
                                        jp, tt, o
                                    )
                                )

                        # ---- causal attention for tq tile j, both head pairs
                        #      outN: [128, pair, 512] normalized attn output;
                        #      partitions 0-63 = head 2p, 64-127 = head 2p+1.
                        outN = wp2.tile([128, 2, 512], F32R, tag="outN")
                        nchunk = 4 * (j + 1)
                        n_it = 2 * nchunk
                        it = 0
                        emitted = 0
                        for hp in range(2):
                            psO = ps_o.tile([65, 1024], F32, tag="po")
                            for i in range(nchunk):
                                dlt = 128 * i - 512 * j
                                dlt = dlt if dlt > 0 else 0
                                pS = ps_s.tile([128, 1024], F32, tag="ps")
                                for hh in range(2):
                                    nc.tensor.matmul(
                                        pS[:, 512 * hh + dlt : 512 * (hh + 1)],
                                        kT[64 * hh : 64 * (hh + 1), hp, 128 * i : 128 * (i + 1)],
                                        qTj[64 * hh : 64 * (hh + 1), hp, dlt:512],
                                        start=True,
                                        stop=True,
                                    )
                                pT = wp3.tile([128, 2, 512], BF16, tag="pT", bufs=3)
                                pSv = pS[:].rearrange("p (h w) -> p h w", h=2)
                                nc.scalar.activation(
                                    pT[:, :, dlt:512], pSv[:, :, dlt:512], AF.Exp, scale=SCALE
                                )
                                if i >= 4 * j:  # diagonal block: causal 0/1 mask
                                    for hh in range(2):
                                        nc.vector.tensor_tensor(
                                            pT[:, hh, dlt : dlt + 128],
                                            pT[:, hh, dlt : dlt + 128],
                                            tri[:],
                                            mybir.AluOpType.mult,
                                        )
                                for hh in range(2):
                                    nc.tensor.matmul(
                                        psO[0:65, 512 * hh + dlt : 512 * (hh + 1)],
                                        v_sb[:, i, 2 * hp + hh, :],
                                        pT[:, hh, dlt:512],
                                        start=(i == 0),
                                        stop=(i == nchunk - 1),
                                        skip_group_check=True,
                                    )
                                it += 1
                                want = len(fillers) * it // n_it
                                while emitted < want:
                                    fillers[emitted]()
                                    emitted += 1

                            # ---- inline normalization: recip of the ones-row
                            #      sums (partition 64), broadcast down 64
                            #      partitions via matmul, multiply during the
                            #      PSUM evacuation.
                            rr = npool.tile([65, 1024], F32R, tag="rr")
                            with nc.allow_low_precision(
                                reason="tf32 reciprocal of softmax denom (~1e-3)"
                            ):
                                nc.vector.reciprocal(rr[64:65, :], psO[64:65, :])
                            for hh in range(2):
                                pB = ps_m.tile([64, 512], F32, tag="m")
                                nc.tensor.matmul(
                                    pB[:],
                                    ones65[64:65, :],
                                    rr[64:65, 512 * hh : 512 * (hh + 1)],
                                    start=True,
                                    stop=True,
                                )
                                pBs = npool.tile([64, 512], F32, tag="pbs")
                                nc.vector.tensor_copy(pBs[:], pB[:])
                                if hh == 0:
                                    nc.vector.tensor_tensor(
                                        outN[0:64, hp, :],
                                        psO[0:64, 0:512],
                                        pBs[:],
                                        mybir.AluOpType.mult,
                                    )
                                else:
                                    stage = npool.tile([64, 512], F32R, tag="stage")
                                    nc.vector.tensor_tensor(
                                        stage[:],
                                        psO[0:64, 512:1024],
                                        pBs[:],
                                        mybir.AluOpType.mult,
                                    )
                                    # odd head of the pair -> partitions 64-127
                                    nc.sync.dma_start(outN[64:128, hp, :], stage[:])

                        # flush any fillers not yet emitted (small j tiles)
                        while emitted < len(fillers):
                            fillers[emitted]()
                            emitted += 1
                        outN_prev = outN
                        qT_cur = qT_nxt

                    # ---- epilogue: out-projection of the last tile
                    for tt in range(4):
                        outproj_unit(NJ - 1, tt, outN_prev)

            if reps == 1:
                body()
            else:
                with tc.For_i(0, reps, 1):
                    body()

    nc.compile()
    return nc


def shard_inputs(x, w_qkv, b_qkv, w_out, T):
    """Build the 8 per-core input maps (core c: batch c//4, head group c%4)."""
    x = np.asarray(x, dtype=np.float32)
    w_qkv = np.asarray(w_qkv, dtype=np.float32)
    b_qkv = np.asarray(b_qkv, dtype=np.float32)
    w_out = np.asarray(w_out, dtype=np.float32)
    in_maps = []
    for c in range(8):
        b, g = c // 4, c % 4
        qcols = slice(4 * g * 64, (4 * g + 4) * 64)
        kcols = slice(D + 4 * g * 64, D + (4 * g + 4) * 64)
        vcols = slice(2 * D + 4 * g * 64, 2 * D + (4 * g + 4) * 64)
        wqk = np.concatenate([w_qkv[:, qcols], w_qkv[:, kcols]], axis=1)  # [D, 512]
        wv = np.ascontiguousarray(w_qkv[:, vcols])  # [D, 256]
        bqk = np.concatenate([b_qkv[qcols], b_qkv[kcols]]).reshape(4, 128).T  # [128,4]
        # [hidden 256] -> [pair p, in-pair ip, hd], stacked so partitions
        # 0-63 = head 2p, 64-127 = head 2p+1
        w4 = w_out[256 * g : 256 * (g + 1), :].reshape(2, 2, 64, D)
        wout = np.ascontiguousarray(w4.transpose(1, 2, 0, 3).reshape(128, 2, D))
        in_maps.append(
            {
                "xt": np.ascontiguousarray(x[b, :T].T),
                "wqk": np.ascontiguousarray(wqk),
                "wv": wv,
                "bqk": np.ascontiguousarray(bqk),
                "wout": wout,
            }
        )
    return in_maps


def assemble_output(results, b_qkv, b_out, w_out, T):
    b_qkv = np.asarray(b_qkv, dtype=np.float32)
    b_out = np.asarray(b_out, dtype=np.float32)
    w_out = np.asarray(w_out, dtype=np.float32)
    extra = b_out + b_qkv[2 * D :] @ w_out  # v-bias folds through softmax
    y = np.zeros((B, T, D), dtype=np.float32)
    for c in range(8):
        y[c // 4] += results[c]["y"]
    y += extra[None, None, :]
    return y


_cache = {}


def kernel(x, w_qkv, b_qkv, w_out, b_out):
    x = np.asarray(x, dtype=np.float32)
    T = x.shape[1]
    if T not in _cache:
        _cache[T] = build(T=T, reps=1)
    nc = _cache[T]
    in_maps = shard_inputs(x, w_qkv, b_qkv, w_out, T)
    for _attempt in range(3):
        res = run_bass_kernel_spmd(nc, in_maps, core_ids=list(range(8)), trace=False)
        y = assemble_output(res.results, b_qkv, b_out, w_out, T)
        if np.isfinite(y).all():  # guard against transient device flakes
            return y
    return y


# revision 11
# speedup vs baseline: 1.3272x; 1.1282x over previous
"""Causal multi-head attention (B=2, T=4096, D=1024, H=16, HD=64) on 8 trn2
NeuronCores.

Sharding: core c handles batch b = c//4 and head group g = c%4 (heads
4g..4g+3).  Each core computes qkv projection for its 4 heads, causal
flash-attention in transposed (S^T) layout, and a partial out-projection
(its 256 columns of the hidden dim).  Host sums the 4 partial outputs per
batch and adds the bias terms.

v2: fully pipelined per-512-query-tile structure.  For each tile j:
qk/v projection -> attention i-loop (QK matmul, exp on ACT, PV matmul
with a ones-column for softmax denominators) -> inline normalization
(reciprocal on PSUM partition 64 + matmul broadcast, fused into the
PSUM-evacuation multiply) -> head-pair restack (odd head of each pair
moved to SBUF partitions 64-127 by DMA) -> out-projection with K=128
-> y DMA.  Everything after the i-loop for tile j overlaps the i-loop
of tile j+1; no DRAM round-trip for the softmax denominators.

Math notes:
  - k-bias kept (cheap), v-bias folded into the host epilogue: softmax
    rows sum to 1, so out += b_v exactly, hence y += b_v @ w_out (+ b_out).
  - softmax computed without max subtraction (scores are O(10) for this
    problem scale; exp stays in fp32 range).
  - softmax denominators come for free as a 65th ones-column in v.
dtypes: q/k path float32r (TF32-like, ~1e-4), P and v bf16, accum fp32.
"""

import ml_dtypes
import numpy as np

import concourse.bass as bass
import concourse.mybir as mybir
import concourse.tile as tile
from concourse import bacc
from concourse.bass_utils import run_bass_kernel_spmd
from concourse.masks import make_upper_triangular

F32 = mybir.dt.float32
F32R = mybir.dt.float32r
BF16 = mybir.dt.bfloat16
AF = mybir.ActivationFunctionType

B, D, H, HD = 2, 1024, 16, 64
NHEADS = 4          # heads per core
SCALE = 1.0 / np.sqrt(HD)


def build(T=4096, reps=1):
    """Build the per-core Bass module. reps>1 wraps the compute in an
    on-device For_i loop (for wall-clock-difference timing)."""
    NJ = T // 512       # tq tiles of 512
    NT = T // 128       # t chunks of 128
    DC = D // 128       # d chunks of 128

    nc = bacc.Bacc("TRN2", target_bir_lowering=False, debug=False, num_devices=8)

    xt_d = nc.dram_tensor("xt", [D, T], BF16, kind="ExternalInput")
    wqk_d = nc.dram_tensor("wqk", [D, 512], BF16, kind="ExternalInput")
    wv_d = nc.dram_tensor("wv", [D, 256], BF16, kind="ExternalInput")
    bqk_d = nc.dram_tensor("bqk", [128, 4], F32, kind="ExternalInput")
    wout_d = nc.dram_tensor("wout", [128, 2, D], BF16, kind="ExternalInput")
    y_d = nc.dram_tensor("y", [T, D], F32, kind="ExternalOutput")

    with tile.TileContext(nc) as tc:
        with (
            tc.tile_pool(name="const", bufs=1) as cp,
            tc.tile_pool(name="persist", bufs=1) as pp,
        ):
            # ---------- constants / weights (outside the timing loop)
            tri32 = cp.tile([128, 128], F32, tag="tri32")
            make_upper_triangular(nc, tri32[:], val=1.0, diag=True)
            tri = cp.tile([128, 128], BF16, tag="tri")
            nc.vector.tensor_copy(tri[:], tri32[:])

            # ones row on partition 64 (for the denominator broadcast matmul)
            ones65 = cp.tile([65, 64], BF16, tag="ones65")
            nc.vector.memset(ones65[64:65, :], 1.0)

            bqk_sb = cp.tile([128, 4], F32, tag="bqk")
            nc.sync.dma_start(bqk_sb[:], bqk_d[:])
            wqk_sb = cp.tile([128, DC, 512], BF16, tag="wqk")
            nc.sync.dma_start(wqk_sb[:], wqk_d.rearrange("(dc p) c -> p dc c", p=128))
            wv_sb = cp.tile([128, DC, 256], BF16, tag="wv")
            nc.sync.dma_start(wv_sb[:], wv_d.rearrange("(dc p) c -> p dc c", p=128))
            wout_sb = cp.tile([128, 2, D], BF16, tag="wout")
            nc.sync.dma_start(wout_sb[:], wout_d[:])

            # ---------- persistent state
            kT = pp.tile([128, 2, T], BF16, tag="kT")          # [qk-col, pair, t]
            v_sb = pp.tile([128, NT, 4, 65], BF16, tag="v")    # [t%128, tchunk, head, hd+one]
            nc.vector.memset(v_sb[:, :, :, 64:65], 1.0)

            def body():
                with (
                    tc.tile_pool(name="work2", bufs=2) as wp2,
                    tc.tile_pool(name="work3", bufs=3) as wp3,
                    tc.tile_pool(name="norm", bufs=2) as npool,
                    tc.tile_pool(name="yout", bufs=3) as ypool,
                    tc.tile_pool(name="pmisc", bufs=2, space="PSUM") as ps_m,
                    tc.tile_pool(name="pscore", bufs=2, space="PSUM") as ps_s,
                    tc.tile_pool(name="pout", bufs=1, space="PSUM") as ps_o,
                ):
                    # ---- filler units: projection of tile jn (emitted
                    #      interleaved into the previous tile's attention
                    #      i-loop so PE fills its ACT-bound slack and the
                    #      next tile's QK is ready immediately)
                    def load_xT(jn):
                        xT = wp2.tile([128, DC, 512], BF16, tag="xT")
                        nc.sync.dma_start(
                            xT[:],
                            xt_d[:, 512 * jn : 512 * jn + 512].rearrange(
                                "(dc p) t -> p dc t", p=128
                            ),
                        )
                        return xT

                    def proj_qk_unit(jn, cc, xT, qT):
                        pqk = ps_m.tile([128, 512], F32, tag="m")
                        for dc in range(DC):
                            nc.tensor.matmul(
                                pqk[:],
                                wqk_sb[:, dc, 128 * cc : 128 * (cc + 1)],
                                xT[:, dc],
                                start=(dc == 0),
                                stop=(dc == DC - 1),
                            )
                        dst = (
                            qT[:, cc]
                            if cc < 2
                            else kT[:, cc - 2, 512 * jn : 512 * jn + 512]
                        )
                        with nc.allow_low_precision(reason="bf16 q/k tiles"):
                            nc.vector.tensor_scalar_add(
                                dst, pqk[:], bqk_sb[:, cc : cc + 1]
                            )

                    def proj_v_unit(jn, ts, xT):
                        pv = ps_m.tile([128, 256], F32, tag="m")
                        for dc in range(DC):
                            nc.tensor.matmul(
                                pv[:],
                                xT[:, dc, 128 * ts : 128 * (ts + 1)],
                                wv_sb[:, dc],
                                start=(dc == 0),
                                stop=(dc == DC - 1),
                            )
                        nc.vector.tensor_copy(
                            v_sb[:, 4 * jn + ts, :, 0:64],
                            pv[:].rearrange("p (h c) -> p h c", h=4),
                        )

                    def outproj_unit(jp, tt, outN_p):
                        ysb = ypool.tile([128, 2, 512], F32, tag="y")
                        for nh in range(2):
                            pY = ps_m.tile([128, 512], F32, tag="m")
                            for p in range(2):
                                nc.tensor.matmul(
                                    pY[:],
                                    outN_p[:, p, 128 * tt : 128 * (tt + 1)],
                                    wout_sb[:, p, 512 * nh : 512 * (nh + 1)],
                                    start=(p == 0),
                                    stop=(p == 1),
                                )
                            nc.vector.tensor_copy(ysb[:, nh, :], pY[:])
                        nc.sync.dma_start(
                            y_d[512 * jp + 128 * tt : 512 * jp + 128 * (tt + 1), :],
                            ysb[:].rearrange("p a b -> p (a b)"),
                        )

                    # ---- prologue: projection for tile 0
                    xT0 = load_xT(0)
                    qT_cur = wp2.tile([128, 2, 512], BF16, tag="qTj")
                    for cc in range(4):
                        proj_qk_unit(0, cc, xT0, qT_cur)
                    for ts in range(4):
                        proj_v_unit(0, ts, xT0)

                    outN_prev = None
                    for j in range(NJ):
                        qTj = qT_cur
                        # build the filler list for this tile's attention:
                        # projection of tile j+1, then out-projection of j-1
                        fillers = []
                        if j + 1 < NJ:
                            xTn = load_xT(j + 1)
                            qT_nxt = wp2.tile([128, 2, 512], BF16, tag="qTj")
                            for cc in range(4):
                                fillers.append(
                                    lambda cc=cc, x=xTn, q=qT_nxt: proj_qk_unit(
                                        j + 1, cc, x, q
                                    )
                                )
                            for ts in range(4):
                                fillers.append(
                                    lambda ts=ts, x=xTn: proj_v_unit(j + 1, ts, x)
                                )
                        else:
                            qT_nxt = None
                        if j >= 1:
                            for tt in range(4):
                                fillers.append(
                                    lambda tt=tt, o=outN_prev, jp=j - 1: outproj_unit(
                                        jp, tt, o
                                    )
                                )

                        # ---- causal attention for tq tile j, both head pairs
                        #      outN: [128, pair, 512] normalized attn output;
                        #      partitions 0-63 = head 2p, 64-127 = head 2p+1.
                        outN = wp2.tile([128, 2, 512], BF16, tag="outN")
                        nchunk = 4 * (j + 1)
                        n_it = 2 * nchunk
                        it = 0
                        emitted = 0
                        for hp in range(2):
                            psO = ps_o.tile([65, 1024], F32, tag="po")
                            for i in range(nchunk):
                                dlt = 128 * i - 512 * j
                                dlt = dlt if dlt > 0 else 0
                                pS = ps_s.tile([128, 1024], F32, tag="ps")
                                for hh in range(2):
                                    nc.tensor.matmul(
                                        pS[:, 512 * hh + dlt : 512 * (hh + 1)],
                                        kT[64 * hh : 64 * (hh + 1), hp, 128 * i : 128 * (i + 1)],
                                        qTj[64 * hh : 64 * (hh + 1), hp, dlt:512],
                                        start=True,
                                        stop=True,
                                    )
                                pT = wp3.tile([128, 2, 512], BF16, tag="pT", bufs=3)
                                pSv = pS[:].rearrange("p (h w) -> p h w", h=2)
                                nc.scalar.activation(
                                    pT[:, :, dlt:512], pSv[:, :, dlt:512], AF.Exp, scale=SCALE
                                )
                                if i >= 4 * j:  # diagonal block: causal 0/1 mask
                                    for hh in range(2):
                                        nc.vector.tensor_tensor(
                                            pT[:, hh, dlt : dlt + 128],
                                            pT[:, hh, dlt : dlt + 128],
                                            tri[:],
                                            mybir.AluOpType.mult,
                                        )
                                for hh in range(2):
                                    nc.tensor.matmul(
                                        psO[0:65, 512 * hh + dlt : 512 * (hh + 1)],
                                        v_sb[:, i, 2 * hp + hh, :],
                                        pT[:, hh, dlt:512],
                                        start=(i == 0),
                                        stop=(i == nchunk - 1),
                                        skip_group_check=True,
                                    )
                                it += 1
                                want = len(fillers) * it // n_it
                                while emitted < want:
                                    fillers[emitted]()
                                    emitted += 1

                            # ---- inline normalization: recip of the ones-row
                            #      sums (partition 64), broadcast down 64
                            #      partitions via matmul, multiply during the
                            #      PSUM evacuation.
                            rr = npool.tile([65, 1024], BF16, tag="rr")
                            with nc.allow_low_precision(
                                reason="bf16 reciprocal of softmax denom"
                            ):
                                nc.vector.reciprocal(rr[64:65, :], psO[64:65, :])
                            for hh in range(2):
                                pB = ps_m.tile([64, 512], F32, tag="m")
                                nc.tensor.matmul(
                                    pB[:],
                                    ones65[64:65, :],
                                    rr[64:65, 512 * hh : 512 * (hh + 1)],
                                    start=True,
                                    stop=True,
                                )
                                pBs = npool.tile([64, 512], F32, tag="pbs")
                                nc.vector.tensor_copy(pBs[:], pB[:])
                                if hh == 0:
                                    with nc.allow_low_precision(
                                        reason="bf16 attn output"
                                    ):
                                        nc.vector.tensor_tensor(
                                            outN[0:64, hp, :],
                                            psO[0:64, 0:512],
                                            pBs[:],
                                            mybir.AluOpType.mult,
                                        )
                                else:
                                    stage = npool.tile([64, 512], BF16, tag="stage")
                                    with nc.allow_low_precision(
                                        reason="bf16 attn output"
                                    ):
                                        nc.vector.tensor_tensor(
                                            stage[:],
                                            psO[0:64, 512:1024],
                                            pBs[:],
                                            mybir.AluOpType.mult,
                                        )
                                    # odd head of the pair -> partitions 64-127
                                    nc.sync.dma_start(outN[64:128, hp, :], stage[:])

                        # flush any fillers not yet emitted (small j tiles)
                        while emitted < len(fillers):
                            fillers[emitted]()
                            emitted += 1
                        outN_prev = outN
                        qT_cur = qT_nxt

                    # ---- epilogue: out-projection of the last tile
                    for tt in range(4):
                        outproj_unit(NJ - 1, tt, outN_prev)

            if reps == 1:
                body()
            else:
                with tc.For_i(0, reps, 1):
                    body()

    nc.compile()
    return nc


def shard_inputs(x, w_qkv, b_qkv, w_out, T):
    """Build the 8 per-core input maps (core c: batch c//4, head group c%4)."""
    x = np.asarray(x, dtype=np.float32)
    w_qkv = np.asarray(w_qkv, dtype=np.float32)
    b_qkv = np.asarray(b_qkv, dtype=np.float32)
    w_out = np.asarray(w_out, dtype=np.float32)
    in_maps = []
    for c in range(8):
        b, g = c // 4, c % 4
        qcols = slice(4 * g * 64, (4 * g + 4) * 64)
        kcols = slice(D + 4 * g * 64, D + (4 * g + 4) * 64)
        vcols = slice(2 * D + 4 * g * 64, 2 * D + (4 * g + 4) * 64)
        wqk = np.concatenate([w_qkv[:, qcols], w_qkv[:, kcols]], axis=1)  # [D, 512]
        wv = np.ascontiguousarray(w_qkv[:, vcols])  # [D, 256]
        bqk = np.concatenate([b_qkv[qcols], b_qkv[kcols]]).reshape(4, 128).T  # [128,4]
        # [hidden 256] -> [pair p, in-pair ip, hd], stacked so partitions
        # 0-63 = head 2p, 64-127 = head 2p+1
        w4 = w_out[256 * g : 256 * (g + 1), :].reshape(2, 2, 64, D)
        wout = np.ascontiguousarray(w4.transpose(1, 2, 0, 3).reshape(128, 2, D))
        bf16 = ml_dtypes.bfloat16
        in_maps.append(
            {
                "xt": np.ascontiguousarray(x[b, :T].T).astype(bf16),
                "wqk": np.ascontiguousarray(wqk).astype(bf16),
                "wv": wv.astype(bf16),
                "bqk": np.ascontiguousarray(bqk),
                "wout": wout.astype(bf16),
            }
        )
    return in_maps


def assemble_output(results, b_qkv, b_out, w_out, T):
    b_qkv = np.asarray(b_qkv, dtype=np.float32)
    b_out = np.asarray(b_out, dtype=np.float32)
    w_out = np.asarray(w_out, dtype=np.float32)
    extra = b_out + b_qkv[2 * D :] @ w_out  # v-bias folds through softmax
    y = np.zeros((B, T, D), dtype=np.float32)
    for c in range(8):
        y[c // 4] += results[c]["y"]
    y += extra[None, None, :]
    return y


_cache = {}


def kernel(x, w_qkv, b_qkv, w_out, b_out):
    x = np.asarray(x, dtype=np.float32)
    T = x.shape[1]
    if T not in _cache:
        _cache[T] = build(T=T, reps=1)
    nc = _cache[T]
    in_maps = shard_inputs(x, w_qkv, b_qkv, w_out, T)
    for _attempt in range(3):
        res = run_bass_kernel_spmd(nc, in_maps, core_ids=list(range(8)), trace=False)
        y = assemble_output(res.results, b_qkv, b_out, w_out, T)
        if np.isfinite(y).all():  # guard against transient device flakes
            return y
    return y
